# revision 15
# baseline (speedup 1.0000x reference)
"""Distributed causal multi-head attention block (LN -> QKV -> causal MHA -> out-proj)
on 8 TRN2 NeuronCores.

Sharding: core c -> batch b = c//4, head group g = c%4 (heads 4g..4g+3).
- LayerNorm duplicated within each quad (cheap, avoids input comm).
- QKV: Megatron column-parallel (each core computes q/k/v for its 4 heads).
- Attention: flash-style, S^T layout ([key j, query i] tiles) so exp(S) feeds
  the PV matmul directly as the moving operand; rowsum via an extra ones
  column in V; causal masking by multiplying exp tiles with precomputed 0/1
  masks (diagonal tiles only); no max-subtraction (values are O(1), exp is
  safe in f32/bf16).
- Ulysses-style switch: one 8-core AllToAll exchanges normalized ctx^T
  token-slices within each quad (cross-quad blocks carry duplicates - A2A
  needs >4 ranks). Out-projection is then token-parallel with the full w_out;
  each core emits y for its 512-token slice of its batch.
All matmuls run in bf16 (4x faster than f32 on the PE array), accumulation f32.
"""

import numpy as np
import ml_dtypes

import concourse.bass as bass
import concourse.mybir as mybir
import concourse.tile as tile
from concourse import bacc, bass_utils

N_CORES = 8
B, N, D = 2, 2048, 1024
HEADS, DH = 16, 64
INNER = HEADS * DH
HPC = 4              # heads per core
NI = 4               # 512-token chunks
IC = 512             # i-chunk width
JT = 128             # j-tile width
NTT = 16             # 128-token tiles
F32 = mybir.dt.float32
BF16 = mybir.dt.bfloat16
AF = mybir.ActivationFunctionType

_CACHE = {}


def _build(has_beta: bool):
    nc = bacc.Bacc("TRN2", target_bir_lowering=False, debug=False,
                   num_devices=N_CORES)

    xT_ext = nc.dram_tensor("xT", [8, 128, N], BF16, kind="ExternalInput")
    csum_ext = nc.dram_tensor("csum", [1, 3 * HPC * DH], BF16,
                              kind="ExternalInput")
    wqkvT_ext = nc.dram_tensor("wqkvT", [8, 128, 3 * HPC * DH], BF16,
                               kind="ExternalInput")
    woutT_ext = nc.dram_tensor("woutT", [8, 128, D], BF16, kind="ExternalInput")
    b_ext = nc.dram_tensor("bvec", [1, D], BF16, kind="ExternalInput")
    qb_ext = nc.dram_tensor("qkvb", [128, 6], BF16, kind="ExternalInput")
    ones_ext = nc.dram_tensor("onesrow", [1, 512], BF16, kind="ExternalInput")
    ident_ext = nc.dram_tensor("ident", [128, 128], BF16, kind="ExternalInput")
    masks_ext = nc.dram_tensor("masks", [4, 128, 2 * IC], BF16,
                               kind="ExternalInput")
    out_ext = nc.dram_tensor("out", [IC, D], F32, kind="ExternalOutput")

    a2a_in = nc.dram_tensor("a2a_in", [8, HPC, DH, IC], BF16)
    a2a_out = nc.dram_tensor("a2a_out", [8, HPC, DH, IC], BF16)

    with tile.TileContext(nc) as tc:
        import contextlib
        ctx = contextlib.ExitStack()
        with ctx:
            consts = ctx.enter_context(tc.tile_pool(name="consts", bufs=1))
            persist = ctx.enter_context(tc.tile_pool(name="persist", bufs=1))
            xnp = ctx.enter_context(tc.tile_pool(name="xnp", bufs=2))
            pexp_pool = ctx.enter_context(tc.tile_pool(name="pexp", bufs=3))
            rsm = ctx.enter_context(tc.tile_pool(name="rsm", bufs=1))
            rbp = ctx.enter_context(tc.tile_pool(name="rbp", bufs=2))
            yp = ctx.enter_context(tc.tile_pool(name="yp", bufs=2))
            drp = ctx.enter_context(tc.tile_pool(name="drp", bufs=4, space="DRAM"))

            # ---- partition id (for A2A receive selection) ----
            with tc.tile_critical():
                pid = nc.sync.partition_id()
                is_lo = pid < 4
                is_hi = pid >= 4

            # ---- constants in ----
            wqkvT_sb = [consts.tile([128, 3 * HPC * DH], BF16, tag=f"wqkvT{k}", name=f"wqkvT{k}")
                        for k in range(8)]
            woutT_sb = [consts.tile([128, D], BF16, tag=f"woutT{k}", name=f"woutT{k}")
                        for k in range(8)]
            for k in range(8):
                nc.sync.dma_start(wqkvT_sb[k], wqkvT_ext[k])
            b_sb = consts.tile([1, D], BF16, tag="bvec")
            if has_beta:
                qb_sb = consts.tile([128, 6], BF16, tag="qkvb")
                nc.sync.dma_start(qb_sb, qb_ext[:, :])
            ones_sb = consts.tile([1, 512], BF16, tag="onesrow")
            nc.sync.dma_start(ones_sb, ones_ext[:, :])
            ident_sb = consts.tile([128, 128], BF16, tag="ident")
            nc.sync.dma_start(ident_sb, ident_ext[:, :])
            masks_sb = consts.tile([128, 4, 2 * IC], BF16, tag="masks")
            for t in range(4):
                nc.sync.dma_start(masks_sb[:, t, :], masks_ext[t])
            eps_sb = consts.tile([128, 1], F32, tag="eps")
            nc.vector.memset(eps_sb, 1e-5)

            # ---- persistent activations ----
            qkvT = [persist.tile([128, N], BF16, tag=f"qkvT{m}", name=f"qkvT{m}") for m in range(6)]
            vnat = persist.tile([128, 16, HPC * (DH + 1)], BF16, tag="vnat")
            ctxn = [persist.tile([DH, N], BF16, tag=f"ctxn{h}", name=f"ctxn{h}")
                    for h in range(HPC)]
            ctxTf = [persist.tile([128, IC], BF16, tag=f"ctxTf{k}", name=f"ctxTf{k}")
                     for k in range(8)]

            nc.vector.memset(vnat, 1.0)  # ones columns survive the V copies

            # ========== Phase 1: load xT; LN stats via PE (ones & x^2) ==========
            xT = [persist.tile([128, N], BF16, tag=f"xT{k}", name=f"xT{k}")
                  for k in range(8)]
            for k in range(8):
                nc.sync.dma_start(xT[k], xT_ext[k])
            csum_sb = consts.tile([1, 3 * HPC * DH], BF16, tag="csum")
            nc.sync.dma_start(csum_sb, csum_ext[:, :])
            ones_col = consts.tile([128, 1], BF16, tag="ones_col")
            nc.vector.memset(ones_col, 1.0)

            rstd_d = nc.dram_tensor("rstd_d", [1, N], F32)
            with tc.tile_pool(name="stps0", bufs=1, space="PSUM") as stps0:
                sum_ps = stps0.tile([1, N], F32, tag="sum")
                sq_ps = stps0.tile([1, N], F32, tag="sq")
                for k in range(8):
                    xsq = xnp.tile([128, N], BF16, tag="xsq")
                    nc.scalar.activation(out=xsq, in_=xT[k], func=AF.Square)
                    for n in range(NI):
                        nc.tensor.matmul(
                            sum_ps[:, IC * n:IC * (n + 1)], lhsT=ones_col,
                            rhs=xT[k][:, IC * n:IC * (n + 1)],
                            start=(k == 0), stop=(k == 7))
                        nc.tensor.matmul(
                            sq_ps[:, IC * n:IC * (n + 1)], lhsT=ones_col,
                            rhs=xsq[:, IC * n:IC * (n + 1)],
                            start=(k == 0), stop=(k == 7))
                # row math on partition 0: mu, var, rstd, -mu (bf16)
                mu_row = persist.tile([1, N], F32, tag="mu_row")
                nc.vector.tensor_scalar_mul(mu_row, sum_ps, 1.0 / D)
                var_row = persist.tile([1, N], F32, tag="var_row")
                nc.vector.tensor_scalar_mul(var_row, sq_ps, 1.0 / D)
            negmu_bf = persist.tile([1, N], BF16, tag="negmu_bf")
            nc.vector.tensor_scalar_mul(negmu_bf, mu_row, -1.0)
            musq_row = persist.tile([1, N], F32, tag="musq_row")
            nc.vector.tensor_mul(musq_row, mu_row, mu_row)
            nc.vector.tensor_sub(var_row, var_row, musq_row)
            # rstd = exp(-0.5*ln(var+eps)); Ln/Exp share one ACT table set
            nc.scalar.activation(out=var_row, in_=var_row, func=AF.Ln,
                                 bias=eps_sb[0:1, :], scale=1.0)
            nc.scalar.activation(out=mu_row, in_=var_row, func=AF.Exp,
                                 scale=-0.5)
            # broadcast rstd to all 128 partitions via DRAM round-trip
            nc.sync.dma_start(rstd_d[:, :], mu_row)
            rstd_bc = persist.tile([128, N], F32, tag="rstd_bc")
            nc.sync.dma_start(
                rstd_bc, bass.AP(tensor=rstd_d, offset=0,
                                 ap=[[0, 128], [1, N]]))

            # ================= Phase 2: QKV projection =================
            with tc.tile_pool(name="qkps", bufs=2, space="PSUM") as qkps, \
                 tc.tile_pool(name="trps", bufs=2, space="PSUM") as trps:
                for n in range(NI):
                    for m in range(6):
                        ps = qkps.tile([128, IC], F32, tag="qk")
                        for k in range(8):
                            nc.tensor.matmul(
                                ps, lhsT=wqkvT_sb[k][:, 128 * m:128 * (m + 1)],
                                rhs=xT[k][:, IC * n:IC * (n + 1)],
                                start=(k == 0), stop=False)
                        nc.tensor.matmul(
                            ps, lhsT=csum_sb[:, 128 * m:128 * (m + 1)],
                            rhs=negmu_bf[:, IC * n:IC * (n + 1)],
                            start=False, stop=True)
                        nc.vector.tensor_mul(
                            qkvT[m][:, IC * n:IC * (n + 1)], ps,
                            rstd_bc[:, IC * n:IC * (n + 1)])
                        if has_beta:
                            nc.vector.tensor_scalar_add(
                                qkvT[m][:, IC * n:IC * (n + 1)],
                                qkvT[m][:, IC * n:IC * (n + 1)],
                                qb_sb[:, m:m + 1])
                    # V of this token chunk back to natural layout
                    for J in range(4 * n, 4 * n + 4):
                        for p in range(2):
                            tp = trps.tile([128, 128], BF16, tag="tr")
                            nc.tensor.transpose(
                                tp, qkvT[4 + p][:, 128 * J:128 * (J + 1)],
                                ident_sb)
                            ba = (DH + 1) * 2 * p
                            bb = (DH + 1) * (2 * p + 1)
                            nc.vector.tensor_copy(
                                vnat[:, J, ba:ba + 64], tp[:, 0:64])
                            nc.vector.tensor_copy(
                                vnat[:, J, bb:bb + 64], tp[:, 64:128])

            # ================= Phase 3: causal flash attention =================
            with tc.tile_pool(name="stps", bufs=2, space="PSUM") as stps, \
                 tc.tile_pool(name="caps", bufs=1, space="PSUM") as caps:
                for I in range(NI):
                    nJ = 4 * I + 4
                    ca = [caps.tile([DH + 1, IC], F32, tag=f"ca{h}", name=f"ca{h}")
                          for h in range(HPC)]
                    for J in range(nJ):
                        for p in range(2):
                            sT = stps.tile([128, 2 * IC], F32, tag="sT")
                            for hl in range(2):
                                nc.tensor.matmul(
                                    sT[:, IC * hl:IC * (hl + 1)],
                                    lhsT=qkvT[2 + p][64 * hl:64 * (hl + 1),
                                                     128 * J:128 * (J + 1)],
                                    rhs=qkvT[p][64 * hl:64 * (hl + 1),
                                                IC * I:IC * (I + 1)],
                                    start=True, stop=True)
                            pexp = pexp_pool.tile([128, 2 * IC], BF16, tag="pexp")
                            nc.scalar.activation(out=pexp, in_=sT, func=AF.Exp)
                            if J >= 4 * I:
                                nc.vector.tensor_mul(
                                    pexp, pexp, masks_sb[:, J - 4 * I, :])
                            for hl in range(2):
                                h = 2 * p + hl
                                vb = (DH + 1) * h
                                nc.tensor.matmul(
                                    ca[h][:, :],
                                    lhsT=vnat[:, J, vb:vb + DH + 1],
                                    rhs=pexp[:, IC * hl:IC * (hl + 1)],
                                    start=(J == 0), stop=(J == nJ - 1))
                    # normalize: ctx rows 0..63 scaled by 1/rowsum (row 64)
                    rrs = []
                    for h in range(HPC):
                        rr = rsm.tile([DH + 1, IC], F32, tag=f"rr{h}",
                                      name=f"rr{h}")
                        nc.scalar.activation(out=rr[DH:DH + 1, :],
                                             in_=ca[h][DH:DH + 1, :],
                                             func=AF.Ln)
                        rrs.append(rr)
                    for h in range(HPC):
                        nc.scalar.activation(out=rrs[h][DH:DH + 1, :],
                                             in_=rrs[h][DH:DH + 1, :],
                                             func=AF.Exp, scale=-1.0)
                    for h in range(HPC):
                        rr = rrs[h]
                        rr_d = drp.tile([1, IC], F32, tag="rrd")
                        nc.sync.dma_start(rr_d, rr[DH:DH + 1, :])
                        rbt = rbp.tile([DH, IC], F32, tag="rbt")
                        nc.sync.dma_start(
                            rbt, bass.AP(tensor=rr_d.tensor, offset=rr_d.offset,
                                         ap=[[0, DH], [1, IC]]))
                        nc.vector.tensor_mul(
                            ctxn[h][:, IC * I:IC * (I + 1)],
                            ca[h][0:DH, :], rbt)
                        for d in (I, 4 + I):
                            nc.sync.dma_start(
                                a2a_in[d, h],
                                ctxn[h][:, IC * I:IC * (I + 1)])

            # ================= Phase 4: A2A ctx exchange =================
            nc.gpsimd.collective_compute(
                "AllToAll", mybir.AluOpType.bypass,
                replica_groups=[list(range(8))],
                ins=[a2a_in.ap().opt()], outs=[a2a_out.ap().opt()])
            for r in range(4):
                for h in range(HPC):
                    dst = ctxTf[2 * r + h // 2][64 * (h % 2):64 * (h % 2) + 64, :]
                    nc.sync.dma_start(dst, a2a_out[r, h], cond=is_lo)
                    nc.sync.dma_start(dst, a2a_out[4 + r, h], cond=is_hi)

            for k in range(8):
                nc.sync.dma_start(woutT_sb[k], woutT_ext[k])
            nc.sync.dma_start(b_sb, b_ext[:, :])

            # ================= Phase 5: out projection =================
            with tc.tile_pool(name="yps", bufs=2, space="PSUM") as yps:
                for t in range(4):
                    for e in range(2):
                        ps = yps.tile([128, IC], F32, tag="y")
                        for kt in range(8):
                            nc.tensor.matmul(
                                ps, lhsT=ctxTf[kt][:, 128 * t:128 * (t + 1)],
                                rhs=woutT_sb[kt][:, IC * e:IC * (e + 1)],
                                start=(kt == 0), stop=False)
                        nc.tensor.matmul(ps, lhsT=ones_sb[:, 0:128],
                                         rhs=b_sb[:, IC * e:IC * (e + 1)],
                                         start=False, stop=True)
                        y_sb = yp.tile([128, IC], F32, tag="ysb")
                        nc.vector.tensor_copy(y_sb, ps)
                        nc.sync.dma_start(
                            out_ext[128 * t:128 * (t + 1), IC * e:IC * (e + 1)],
                            y_sb)
    nc.compile()
    return nc


def _get(has_beta: bool):
    if has_beta not in _CACHE:
        _CACHE[has_beta] = _build(has_beta)
    return _CACHE[has_beta]


def _prep_in_maps(x, ln_gamma, ln_beta, w_qkv, w_out, b_out):
    bf = ml_dtypes.bfloat16
    scale = DH ** -0.5
    wq = w_qkv * ln_gamma[None, :]          # fold gamma into the projection
    qkv_bias = (w_qkv @ ln_beta).astype(np.float32)   # beta contribution
    has_beta = bool(np.any(ln_beta != 0.0))

    masks = np.zeros((4, 128, 2 * IC), np.float32)
    jj = np.arange(128)[:, None]
    ii = np.arange(IC)[None, :]
    for t in range(4):
        m = (jj + 128 * t <= ii).astype(np.float32)
        masks[t, :, 0:IC] = m
        masks[t, :, IC:] = m
    masks = masks.astype(bf)

    ones_row = np.ones((1, 512), bf)
    ident = np.eye(128, dtype=np.float32).astype(bf)
    woutT = np.ascontiguousarray(w_out.T).reshape(8, 128, D).astype(bf)
    b_vec = b_out.reshape(1, D).astype(bf)

    in_maps = []
    for c in range(N_CORES):
        b, g = c // 4, c % 4
        rows = []
        for part in range(3):           # q, k, v rows for heads 4g..4g+3
            lo = part * INNER + 256 * g
            rows.append(wq[lo:lo + 256])
        w_core = np.concatenate(rows, axis=0)          # [768, 1024]
        w_core = w_core.copy()
        w_core[0:256] *= scale                         # fold q scale
        qb_core = np.concatenate(
            [qkv_bias[part * INNER + 256 * g: part * INNER + 256 * g + 256]
             for part in range(3)])
        qb_core = qb_core.copy()
        qb_core[0:256] *= scale
        wqkvT = np.ascontiguousarray(w_core.T).reshape(8, 128, 768).astype(bf)
        in_maps.append({
            "xT": np.ascontiguousarray(x[b].T).astype(bf).reshape(8, 128, N),
            "wqkvT": wqkvT,
            "woutT": woutT,
            "bvec": b_vec,
            "qkvb": np.ascontiguousarray(qb_core.reshape(6, 128).T).astype(bf),
            "csum": w_core.sum(axis=1).reshape(1, 768).astype(bf),
            "onesrow": ones_row,
            "ident": ident,
            "masks": masks,
        })
    return in_maps, has_beta


def kernel(x, ln_gamma, ln_beta, w_qkv, w_out, b_out, _trace=False,
           _trace_kwargs=None):
    x = np.asarray(x, np.float32)
    ln_gamma = np.asarray(ln_gamma, np.float32)
    ln_beta = np.asarray(ln_beta, np.float32)
    w_qkv = np.asarray(w_qkv, np.float32)
    w_out = np.asarray(w_out, np.float32)
    b_out = np.asarray(b_out, np.float32)

    in_maps, has_beta = _prep_in_maps(x, ln_gamma, ln_beta, w_qkv, w_out, b_out)
    nc = _get(has_beta)
    kw = {}
    if _trace:
        kw = dict(trace=True, **(_trace_kwargs or {}))
    res = bass_utils.run_bass_kernel_spmd(
        nc, in_maps, core_ids=list(range(N_CORES)), **kw)
    out = np.empty((B, N, D), np.float32)
    for c in range(N_CORES):
        b, g = c // 4, c % 4
        out[b, IC * g:IC * (g + 1), :] = res.results[c]["out"]
    if _trace:
        return out, res
    return out
